# revision 7
# baseline (speedup 1.0000x reference)
"""Trainium2 Bass kernel for ContextQueryAttention (BiDAF-style trilinear attention).

Math (per batch):
  S = C@w1 + (Q@w2)^T + (C*w3)@Q^T          [n, m]
  S_row = softmax_m(S); S_col = softmax_n(S)
  A = S_row @ Q
  B = S_row @ (S_col^T @ C)                  (reassociated: avoids [n,n] intermediate)
  out = [C, A, C*A, C*B]                     [n, 4d]

v2 (bf16 I/O): the problem is memory-bound; the fp32 version sat at the DMA
roofline (~20.5 MB/core/rep @ ~430 GB/s ~= 48 us). This version cuts HBM
traffic ~2.5x:
  - C, Q are downcast to bf16 on the HOST; the device loads bf16.
  - The C output block is NOT stored by the device -- the host already has C
    and assembles out[:, :, 0:128] = C itself (exact fp32).
  - The device stores only [A | C*A | C*B] in bf16 (0.75 MB/batch vs 2 MB);
    the host upcasts. Measured numeric impact (faithful numpy sim of this
    dataflow): rel_err ~= 8e-4 vs the 2e-2 gate.
  - All matmuls run in bf16 (full-rate PE, FWL weight loads); accumulation
    stays fp32 in PSUM, softmax denominators fp32 on-chip.
  - E = exp(S) computed in BOTH orientations by matmul + exp as before:
      E^T[j,i] = exp(sum_d C[i,d]*(Q[j,d]w3[d]+w1[d]) + Qw2[j])   (bias = per-partition)
      Enat[i,j] = exp(sum_d ...)  (drops the exp(Qw2[j]) column factor -- cancels
        in the column softmax since csn is derived consistently via gneg)
  - n is indexed as n = 8*p + c (p = SBUF partition, c = chunk): C load and the
    output store are fully contiguous per partition (2KB / 6KB lines).
  - Per chunk, ONE N=257 bf16 matmul computes [E@Q | E@T2 | rowsum(E)] against
    [Q | T2 | 1]; col sums come fused from the exp-activation's accum_out.
  - Sharding: data-parallel over batch, 8 batches per core, no communication.
"""
import numpy as np
import ml_dtypes

B, N, M, D = 64, 1024, 128, 128
NCORES = 8
BPC = B // NCORES      # batches per core
NCH = N // 128         # 128-row chunks per batch

_CACHE = {}


def _build_program(nreps=1):
    import concourse.tile as tile
    from concourse import bacc, masks, mybir

    fp32 = mybir.dt.float32
    bf16 = mybir.dt.bfloat16
    AL = mybir.AluOpType
    AF = mybir.ActivationFunctionType

    nc = bacc.Bacc("TRN2", target_bir_lowering=False, debug=False, num_devices=NCORES)
    C_d = nc.dram_tensor("Cin", [BPC, N, D], bf16, kind="ExternalInput")
    Q_d = nc.dram_tensor("Qin", [BPC, M, D], bf16, kind="ExternalInput")
    W_d = nc.dram_tensor("Win", [3 * D], fp32, kind="ExternalInput")
    O_d = nc.dram_tensor("Out", [BPC, N, 3 * D], bf16, kind="ExternalOutput")

    with tile.TileContext(nc) as tc:
        with (
            tc.tile_pool(name="const", bufs=1) as constp,
            tc.tile_pool(name="small", bufs=2) as smallp,
            tc.tile_pool(name="cbuf", bufs=3) as cbufp,
            tc.tile_pool(name="ctp", bufs=2) as ctp,
            tc.tile_pool(name="ebuf", bufs=2) as ebufp,
            tc.tile_pool(name="obuf", bufs=3) as obufp,
            tc.tile_pool(name="psb", bufs=2, space="PSUM") as psbig,
            tc.tile_pool(name="ps24", bufs=3, space="PSUM") as ps24p,
            tc.tile_pool(name="pss", bufs=1, space="PSUM") as pssmall,
        ):
            identb = constp.tile([128, 128], bf16)
            masks.make_identity(nc, identb[:])
            w_all = constp.tile([128, 3], fp32)
            nc.gpsimd.dma_start(w_all[:], W_d.ap().rearrange("(k p) -> p k", k=3))
            w1c, w2c, w3c = w_all[:, 0:1], w_all[:, 1:2], w_all[:, 2:3]

            def load_inputs(bi):
                """Issue batch bi's input DMAs (prefetched ahead of compute)."""
                b = bi % BPC
                C_sb = cbufp.tile([128, NCH, 128], bf16, tag="csb")
                nc.gpsimd.dma_start(
                    C_sb[:], C_d.ap()[b].rearrange("(p c) d -> p c d", c=NCH)
                )
                # qt2 = [Q | T2 | 1]: Q lands straight here; T2 written later.
                qt2 = cbufp.tile([128, 257], bf16, tag="qt2")
                nc.gpsimd.dma_start(qt2[:, 0:128], Q_d.ap()[b])
                nc.gpsimd.memset(qt2[:, 256:257], 1.0)
                return C_sb, qt2

            TOT = BPC * nreps
            pre = load_inputs(0)
            for bi in range(TOT):
                b = bi % BPC
                C_sb, qt2 = pre
                if bi + 1 < TOT:
                    pre = load_inputs(bi + 1)

                # ---- Q^T, Wmat^T = Q^T*w3 + w1, Qw2
                # transpose as a plain matmul: lhsT.T @ I (fp32 PSUM out)
                qt_ps = pssmall.tile([128, 128], fp32, tag="ps_small")
                nc.tensor.matmul(qt_ps[:], qt2[:, 0:128], identb[:])
                QT = smallp.tile([128, 128], fp32, tag="qt")
                nc.scalar.copy(QT[:], qt_ps[:])
                Wm = smallp.tile([128, 128], bf16, tag="wm")
                nc.vector.tensor_scalar(Wm[:], QT[:], w3c, w1c, AL.mult, AL.add)
                qw2_ps = pssmall.tile([128, 1], fp32, tag="ps_small")
                nc.tensor.matmul(qw2_ps[:], QT[:], w2c)
                qw2 = smallp.tile([128, 1], fp32, tag="qw2")
                nc.scalar.copy(qw2[:], qw2_ps[:])
                gneg = smallp.tile([128, 1], fp32, tag="gneg")
                nc.scalar.activation(gneg[:], qw2_ps[:], AF.Exp, scale=-1.0)

                # ---- C^T via PE transpose (bf16 full-rate)
                ct_ps = psbig.tile([128, NCH, 128], fp32, tag="ps_big")
                for c in range(NCH):
                    nc.tensor.matmul(ct_ps[:, c, :], C_sb[:, c, :], identb[:])
                CT = ctp.tile([128, NCH, 128], bf16, tag="ct")  # [d, c, p]
                nc.vector.tensor_copy(
                    CT[:].rearrange("d c p -> d (c p)"),
                    ct_ps[:].rearrange("d c p -> d (c p)"),
                )
                CT_flat = CT[:].rearrange("d c p -> d (c p)")

                # ---- E^T = exp(Wmat @ C^T + Qw2), accum -> cs   [j, (c p)]
                st_ps = psbig.tile([128, NCH, 128], fp32, tag="ps_big")
                st_flat = st_ps[:].rearrange("m c p -> m (c p)")
                nc.tensor.matmul(st_flat[:, 0:512], Wm[:], CT_flat[:, 0:512])
                nc.tensor.matmul(st_flat[:, 512:1024], Wm[:], CT_flat[:, 512:1024])
                ET = ebufp.tile([128, NCH, 128], bf16, tag="et")  # [j, c, p]
                cs = smallp.tile([128, 1], fp32, tag="cs")
                nc.scalar.activation(
                    ET[:].rearrange("m c p -> m (c p)"),
                    st_flat,
                    AF.Exp,
                    bias=qw2[:],
                    accum_out=cs[:],
                )

                # ---- Enat = exp(C @ Wmat^T)  [p, c, j]
                sn_ps = psbig.tile([128, NCH, 128], fp32, tag="ps_big")
                for c in range(NCH):
                    nc.tensor.matmul(sn_ps[:, c, :], CT[:, c, :], Wm[:])
                EN = ebufp.tile([128, NCH, 128], bf16, tag="en")
                nc.scalar.activation(
                    EN[:].rearrange("p c j -> p (c j)"),
                    sn_ps[:].rearrange("p c j -> p (c j)"),
                    AF.Exp,
                )

                # ---- T2 = (Enat^T @ C) / csnat  -> qt2[:, 128:256]
                ec_ps = pssmall.tile([128, 128], fp32, tag="ps_small")
                for c in range(NCH):
                    nc.tensor.matmul(
                        ec_ps[:], EN[:, c, :], C_sb[:, c, :],
                        start=(c == 0), stop=(c == NCH - 1),
                    )
                csn = smallp.tile([128, 1], fp32, tag="csn")
                nc.vector.tensor_mul(csn[:], cs[:], gneg[:])
                rcs = smallp.tile([128, 1], fp32, tag="rcs")
                nc.vector.reciprocal(rcs[:], csn[:])
                nc.vector.tensor_scalar_mul(qt2[:, 128:256], ec_ps[:], rcs[:])

                # ---- per chunk: one N=257 bf16 matmul [EQ | ET2 | rs], epilogue
                o_big = obufp.tile([128, NCH, 384], bf16, tag="obig")
                rrs = smallp.tile([128, NCH], fp32, tag="rrs")
                for c in range(NCH):
                    p24 = ps24p.tile([128, 257], fp32, tag="ps24")
                    nc.tensor.matmul(p24[:], ET[:, c, :], qt2[:])
                    rr = rrs[:, c : c + 1]
                    nc.vector.reciprocal(rr, p24[:, 256:257])
                    nc.vector.tensor_scalar_mul(
                        o_big[:, c, 0:128], p24[:, 0:128], rr
                    )
                    nc.vector.tensor_mul(
                        o_big[:, c, 128:256], o_big[:, c, 0:128], C_sb[:, c, :]
                    )
                    nc.vector.scalar_tensor_tensor(
                        o_big[:, c, 256:384], p24[:, 128:256], rr, C_sb[:, c, :],
                        AL.mult, AL.mult,
                    )
                # ---- single contiguous 0.75MB store
                nc.sync.dma_start(
                    O_d.ap()[b].rearrange("(p c) e -> p c e", c=NCH), o_big[:]
                )

    nc.compile()
    return nc


def make_in_maps(C, Q, W):
    bf = ml_dtypes.bfloat16
    Cb = np.ascontiguousarray(C, dtype=np.float32).astype(bf)
    Qb = np.ascontiguousarray(Q, dtype=np.float32).astype(bf)
    Wf = np.ascontiguousarray(W, dtype=np.float32)
    return [
        {
            "Cin": Cb[i * BPC : (i + 1) * BPC],
            "Qin": Qb[i * BPC : (i + 1) * BPC],
            "Win": Wf,
        }
        for i in range(NCORES)
    ]


def kernel(C, Q, W):
    from concourse.bass_utils import run_bass_kernel_spmd

    if "nc" not in _CACHE:
        _CACHE["nc"] = _build_program()
    nc = _CACHE["nc"]

    in_maps = make_in_maps(C, Q, W)
    res = run_bass_kernel_spmd(nc, in_maps, core_ids=list(range(NCORES)))
    _CACHE["last_result"] = res
    out = np.empty((B, N, 4 * D), dtype=np.float32)
    out[:, :, 0:D] = np.asarray(C, dtype=np.float32)
    blocks = np.concatenate([r["Out"] for r in res.results], axis=0)
    out[:, :, D:] = blocks  # bf16 -> fp32 upcast on assignment
    return out


# revision 21
# speedup vs baseline: 1.5472x; 1.5472x over previous
"""Trainium2 Bass kernel for ContextQueryAttention (BiDAF-style trilinear attention).

Math (per batch):
  S = C@w1 + (Q@w2)^T + (C*w3)@Q^T          [n, m]
  S_row = softmax_m(S); S_col = softmax_n(S)
  A = S_row @ Q
  B = S_row @ (S_col^T @ C)                  (reassociated: avoids [n,n] intermediate)
  out = [C, A, C*A, C*B]                     [n, 4d]

v9: single-exp bf16 pipeline with host-side transposes.
  - Host ships [C | 1] (natural), [C^T | Q^T] (packed per d-partition), and
    [g | Q] with g = exp(Q@w2) -- all bf16. Device stores RAW [A' | B' | rs]
    bf16; host normalizes (A = A'/rs, B = B'/rs) and assembles
    out = [C, A, C*A, C*B] fp32.
  - ONE exp per batch: Enat = exp(C @ Wm). The row-side matrix E^T is the PE
    transpose of Enat with the exp(qw2) column factor folded in by the DVE
    scale-drain (2-byte 2x mode): ET[j,n] = Enat^T[j,n] * g[j]. Numerator and
    denominator (the matmul ones-column) use the same rounded ET, so the
    factor cancels consistently in the row softmax.
  - Column chain: ec = Enat^T @ [C|1] gives T2' and the column sums together;
    T2 = T2'/csn. No dependence on the row side at all.
  - Per chunk ONE N=257 bf16 matmul [E@Q | E@T2 | rowsum]; drains are plain
    257-col copies, lag-2, 4 on ACT / 4 on DVE.
  - PSUM: sn 2x2 banks + {ec, E^T-transpose, p24} rotation 4 banks = 8.
"""
import numpy as np
import ml_dtypes

B, N, M, D = 64, 1024, 128, 128
NCORES = 8
BPC = B // NCORES      # batches per core
NCH = N // 128         # 128-row chunks per batch

_CACHE = {}


def _build_program(nreps=1):
    import concourse.tile as tile
    from concourse import bacc, masks, mybir

    fp32 = mybir.dt.float32
    bf16 = mybir.dt.bfloat16
    AL = mybir.AluOpType
    AF = mybir.ActivationFunctionType

    nc = bacc.Bacc("TRN2", target_bir_lowering=False, debug=False, num_devices=NCORES)
    C_d = nc.dram_tensor("Cin", [BPC, N, D + 1], bf16, kind="ExternalInput")   # [C | 1]
    T_d = nc.dram_tensor("Tin", [BPC, D, N + M], bf16, kind="ExternalInput")   # [C^T | Q^T]
    Q_d = nc.dram_tensor("Qin", [BPC, M, D + 1], bf16, kind="ExternalInput")   # [g | Q]
    W_d = nc.dram_tensor("Win", [3 * D], fp32, kind="ExternalInput")
    O_d = nc.dram_tensor("Out", [BPC, N, 257], bf16, kind="ExternalOutput")

    ACT_DRAINS = (1, 3, 5, 7)

    with tile.TileContext(nc) as tc:
        with (
            tc.tile_pool(name="const", bufs=1) as constp,
            tc.tile_pool(name="small", bufs=2) as smallp,
            tc.tile_pool(name="cbuf", bufs=3) as cbufp,
            tc.tile_pool(name="ebuf", bufs=2) as ebufp,
            tc.tile_pool(name="obuf", bufs=3) as obufp,
            tc.tile_pool(name="psb", bufs=2, space="PSUM") as psbig,
            tc.tile_pool(name="ps24", bufs=4, space="PSUM") as ps24p,
        ):
            identb = constp.tile([128, 128], bf16)
            masks.make_identity(nc, identb[:])
            w_all = constp.tile([128, 3], fp32)
            nc.gpsimd.dma_start(w_all[:], W_d.ap().rearrange("(k p) -> p k", k=3))
            w1c, w3c = w_all[:, 0:1], w_all[:, 2:3]

            def load_inputs(bi):
                b = bi % BPC
                C_sb = cbufp.tile([128, NCH, D + 1], bf16, tag="csb")  # [C | 1]
                nc.scalar.dma_start(
                    C_sb[:], C_d.ap()[b].rearrange("(p c) d -> p c d", c=NCH)
                )
                CTQ = cbufp.tile([128, N + M], bf16, tag="ctq")  # [C^T | Q^T] on d
                nc.scalar.dma_start(CTQ[:], T_d.ap()[b])
                # qt2 cols: [g | Q(1:129) | T2(129:257) | ones(257)]
                qt2 = cbufp.tile([128, 258], bf16, tag="qt2")
                nc.scalar.dma_start(qt2[:, 0:129], Q_d.ap()[b])
                nc.gpsimd.memset(qt2[:, 257:258], 1.0)
                return C_sb, CTQ, qt2

            def make_wm(CTQ):
                Wm = smallp.tile([128, 128], bf16, tag="wm")
                nc.vector.tensor_scalar(
                    Wm[:], CTQ[:, N : N + M], w3c, w1c, AL.mult, AL.add
                )
                return Wm

            TOT = BPC * nreps
            pre_io = load_inputs(0)
            cur = (pre_io, make_wm(pre_io[1]))
            for bi in range(TOT):
                b = bi % BPC
                (C_sb, CTQ, qt2), Wm = cur
                nxt_io = load_inputs(bi + 1) if bi + 1 < TOT else None

                # ---- Enat = exp(C @ Wm) -- the ONLY exp  [p, c, j]
                sn_ps = psbig.tile([128, NCH, 128], fp32, tag="stsn")
                for c in range(NCH):
                    nc.tensor.matmul(
                        sn_ps[:, c, :], CTQ[:, c * 128 : (c + 1) * 128], Wm[:]
                    )
                EN = ebufp.tile([128, NCH, 128], bf16, tag="en")
                nc.scalar.activation(
                    EN[:].rearrange("p c j -> p (c j)"),
                    sn_ps[:].rearrange("p c j -> p (c j)"),
                    AF.Exp,
                )

                # ---- column chain: ec = Enat^T @ [C|1] -> [T2' | csn], then
                # T2 = T2'/csn -> qt2[:, 129:257]
                ec_ps = ps24p.tile([128, D + 1], fp32, tag="ps24")
                for c in range(NCH):
                    nc.tensor.matmul(
                        ec_ps[:], EN[:, c, :], C_sb[:, c, :],
                        start=(c == 0), stop=(c == NCH - 1),
                    )
                rcs = smallp.tile([128, 1], fp32, tag="rcs")
                nc.vector.reciprocal(rcs[:], ec_ps[:, D : D + 1])
                nc.vector.tensor_scalar_mul(qt2[:, 129:257], ec_ps[:, 0:D], rcs[:])

                # ---- E^T = Enat^T * g  via PE transpose + DVE scale-drain
                et_ps = ps24p.tile([128, NCH, 128], bf16, tag="ps24")  # [j, c, p]
                for c in range(NCH):
                    nc.tensor.transpose(et_ps[:, c, :], EN[:, c, :], identb[:])
                gf = smallp.tile([128, 1], fp32, tag="gf")
                nc.scalar.copy(gf[:], qt2[:, 0:1])
                ET = ebufp.tile([128, NCH, 128], bf16, tag="et")  # [j, c, p]
                nc.vector.tensor_scalar_mul(
                    ET[:].rearrange("m c p -> m (c p)"),
                    et_ps[:].rearrange("m c p -> m (c p)"),
                    gf[:],
                )

                # ---- next batch's Wm (its inputs are already loading)
                if nxt_io is not None:
                    cur = (nxt_io, make_wm(nxt_io[1]))

                # ---- chunk loop: one N=257 bf16 matmul [EQ | ET2 | rs] per
                # chunk; plain 257-col copy drains, lag 2, 4 ACT / 4 DVE
                o_big = obufp.tile([128, NCH, 257], bf16, tag="obig")
                p24s = [None] * NCH

                def drain(c):
                    if c in ACT_DRAINS:
                        nc.scalar.copy(o_big[:, c, :], p24s[c][:, 0:257])
                    else:
                        nc.vector.tensor_copy(o_big[:, c, :], p24s[c][:, 0:257])

                for c in range(NCH):
                    p24s[c] = ps24p.tile(
                        [128, 257], fp32, tag="ps24", name=f"p24_{bi}_{c}"
                    )
                    nc.tensor.matmul(p24s[c][:], ET[:, c, :], qt2[:, 1:258])
                    if c >= 2:
                        drain(c - 2)
                drain(NCH - 2)
                drain(NCH - 1)

                # ---- single contiguous ~0.5MB store
                nc.sync.dma_start(
                    O_d.ap()[b].rearrange("(p c) e -> p c e", c=NCH), o_big[:]
                )

    nc.compile()
    return nc


def make_in_maps(C, Q, W):
    bf = ml_dtypes.bfloat16
    Cf = np.ascontiguousarray(C, dtype=np.float32)
    Qf = np.ascontiguousarray(Q, dtype=np.float32)
    Wf = np.ascontiguousarray(W, dtype=np.float32)
    g = np.exp(Qf @ Wf[D : 2 * D])                 # [B, M] fp32 on host
    Qcat = np.concatenate([g[:, :, None], Qf], axis=2).astype(bf)  # [B, M, D+1]
    Ccat = np.concatenate([Cf, np.ones((B, N, 1), np.float32)], axis=2).astype(bf)
    # [C^T | Q^T]: Tin[b, d, c*128+p] = C[b, 8p+c, d]; Tin[b, d, N+j] = Q[b, j, d]
    Cb = Cf.astype(bf).reshape(B, 128, NCH, D)     # [b, p, c, d]
    CT = np.ascontiguousarray(Cb.transpose(0, 3, 2, 1)).reshape(B, D, N)
    QT = np.ascontiguousarray(Qf.astype(bf).transpose(0, 2, 1))  # [b, d, j]
    Tin = np.concatenate([CT, QT], axis=2)         # [B, D, N+M] bf16
    return [
        {
            "Cin": Ccat[i * BPC : (i + 1) * BPC],
            "Tin": Tin[i * BPC : (i + 1) * BPC],
            "Qin": Qcat[i * BPC : (i + 1) * BPC],
            "Win": Wf,
        }
        for i in range(NCORES)
    ]


def kernel(C, Q, W):
    from concourse.bass_utils import run_bass_kernel_spmd

    if "nc" not in _CACHE:
        _CACHE["nc"] = _build_program()
    nc = _CACHE["nc"]

    in_maps = make_in_maps(C, Q, W)
    res = run_bass_kernel_spmd(nc, in_maps, core_ids=list(range(NCORES)))
    _CACHE["last_result"] = res
    Cf = np.asarray(C, dtype=np.float32)
    ab = np.concatenate([r["Out"] for r in res.results], axis=0)  # [B, N, 257] bf16
    ab = ab.astype(np.float32)
    rs = ab[:, :, 256:257]
    A = ab[:, :, 0:D] / rs
    Bm = ab[:, :, D : 2 * D] / rs
    out = np.empty((B, N, 4 * D), dtype=np.float32)
    out[:, :, 0:D] = Cf
    out[:, :, D : 2 * D] = A
    out[:, :, 2 * D : 3 * D] = Cf * A
    out[:, :, 3 * D : 4 * D] = Cf * Bm
    return out


# revision 24
# speedup vs baseline: 1.7029x; 1.1006x over previous
"""Trainium2 Bass kernel for ContextQueryAttention (BiDAF-style trilinear attention).

Math (per batch):
  S = C@w1 + (Q@w2)^T + (C*w3)@Q^T          [n, m]
  S_row = softmax_m(S); S_col = softmax_n(S)
  A = S_row @ Q
  B = S_row @ (S_col^T @ C)                  (reassociated: avoids [n,n] intermediate)
  out = [C, A, C*A, C*B]                     [n, 4d]

v9: single-exp bf16 pipeline with host-side transposes.
  - Host ships [C | 1] (natural), [C^T | Q^T] (packed per d-partition), and
    [g | Q] with g = exp(Q@w2) -- all bf16. Device stores RAW [A' | B' | rs]
    bf16; host normalizes (A = A'/rs, B = B'/rs) and assembles
    out = [C, A, C*A, C*B] fp32.
  - ONE exp per batch: Enat = exp(C @ Wm). The row-side matrix E^T is the PE
    transpose of Enat with the exp(qw2) column factor folded in by the DVE
    scale-drain (2-byte 2x mode): ET[j,n] = Enat^T[j,n] * g[j]. Numerator and
    denominator (the matmul ones-column) use the same rounded ET, so the
    factor cancels consistently in the row softmax.
  - Column chain: ec = Enat^T @ [C|1] gives T2' and the column sums together;
    T2 = T2'/csn. No dependence on the row side at all.
  - Per chunk ONE N=257 bf16 matmul [E@Q | E@T2 | rowsum]; drains are plain
    257-col copies, lag-2, 4 on ACT / 4 on DVE.
  - PSUM: sn 2x2 banks + {ec, E^T-transpose, p24} rotation 4 banks = 8.
"""
import numpy as np
import ml_dtypes

B, N, M, D = 64, 1024, 128, 128
NCORES = 8
BPC = B // NCORES      # batches per core
NCH = N // 128         # 128-row chunks per batch

_CACHE = {}


def _build_program(nreps=1):
    import concourse.tile as tile
    from concourse import bacc, masks, mybir

    fp32 = mybir.dt.float32
    bf16 = mybir.dt.bfloat16
    AL = mybir.AluOpType
    AF = mybir.ActivationFunctionType

    nc = bacc.Bacc("TRN2", target_bir_lowering=False, debug=False, num_devices=NCORES)
    C_d = nc.dram_tensor("Cin", [BPC, N, D + 1], bf16, kind="ExternalInput")   # [C | 1]
    T_d = nc.dram_tensor("Tin", [BPC, D, N + M], bf16, kind="ExternalInput")   # [C^T | Q^T]
    Q_d = nc.dram_tensor("Qin", [BPC, M, D + 1], bf16, kind="ExternalInput")   # [g | Q]
    W_d = nc.dram_tensor("Win", [3 * D], fp32, kind="ExternalInput")
    O_d = nc.dram_tensor("Out", [BPC, N, 257], bf16, kind="ExternalOutput")

    ACT_DRAINS = (1, 3, 5, 7)

    with tile.TileContext(nc) as tc:
        with (
            tc.tile_pool(name="const", bufs=1) as constp,
            tc.tile_pool(name="small", bufs=2) as smallp,
            tc.tile_pool(name="cbuf", bufs=3) as cbufp,
            tc.tile_pool(name="ebuf", bufs=2) as ebufp,
            tc.tile_pool(name="obuf", bufs=3) as obufp,
            tc.tile_pool(name="psb", bufs=2, space="PSUM") as psbig,
            tc.tile_pool(name="ps24", bufs=4, space="PSUM") as ps24p,
        ):
            identb = constp.tile([128, 128], bf16)
            masks.make_identity(nc, identb[:])
            w_all = constp.tile([128, 3], fp32)
            nc.gpsimd.dma_start(w_all[:], W_d.ap().rearrange("(k p) -> p k", k=3))
            w1c, w3c = w_all[:, 0:1], w_all[:, 2:3]

            def load_inputs(bi):
                b = bi % BPC
                C_sb = cbufp.tile([128, NCH, D + 1], bf16, tag="csb")  # [C | 1]
                nc.scalar.dma_start(
                    C_sb[:], C_d.ap()[b].rearrange("(p c) d -> p c d", c=NCH)
                )
                CTQ = cbufp.tile([128, N + M], bf16, tag="ctq")  # [C^T | Q^T] on d
                nc.scalar.dma_start(CTQ[:], T_d.ap()[b])
                # qt2 cols: [g | Q(1:129) | T2(129:257) | ones(257)]
                qt2 = cbufp.tile([128, 258], bf16, tag="qt2")
                nc.scalar.dma_start(qt2[:, 0:129], Q_d.ap()[b])
                nc.gpsimd.memset(qt2[:, 257:258], 1.0)
                return C_sb, CTQ, qt2

            def make_wm(CTQ):
                Wm = smallp.tile([128, 128], bf16, tag="wm")
                nc.vector.tensor_scalar(
                    Wm[:], CTQ[:, N : N + M], w3c, w1c, AL.mult, AL.add
                )
                return Wm

            TOT = BPC * nreps
            pre_io = load_inputs(0)
            cur = (pre_io, make_wm(pre_io[1]))
            for bi in range(TOT):
                b = bi % BPC
                (C_sb, CTQ, qt2), Wm = cur
                nxt_io = load_inputs(bi + 1) if bi + 1 < TOT else None

                # ---- Enat = exp(C @ Wm) -- the ONLY exp  [p, c, j]
                sn_ps = psbig.tile([128, NCH, 128], fp32, tag="stsn")
                for c in range(NCH):
                    nc.tensor.matmul(
                        sn_ps[:, c, :], CTQ[:, c * 128 : (c + 1) * 128], Wm[:]
                    )
                EN = ebufp.tile([128, NCH, 128], bf16, tag="en")
                nc.scalar.activation(
                    EN[:].rearrange("p c j -> p (c j)"),
                    sn_ps[:].rearrange("p c j -> p (c j)"),
                    AF.Exp,
                )

                # ---- column chain: ec = Enat^T @ [C|1] -> [T2' | csn], then
                # T2 = T2'/csn -> qt2[:, 129:257]
                ec_ps = ps24p.tile([128, D + 1], fp32, tag="ps24")
                for c in range(NCH):
                    nc.tensor.matmul(
                        ec_ps[:], EN[:, c, :], C_sb[:, c, :],
                        start=(c == 0), stop=(c == NCH - 1),
                    )
                rcs = smallp.tile([128, 1], fp32, tag="rcs")
                nc.vector.reciprocal(rcs[:], ec_ps[:, D : D + 1])
                nc.vector.tensor_scalar_mul(qt2[:, 129:257], ec_ps[:, 0:D], rcs[:])

                # ---- E^T = Enat^T * g  via PE transpose + DVE scale-drain
                et_ps = ps24p.tile([128, NCH, 128], bf16, tag="ps24")  # [j, c, p]
                for c in range(NCH):
                    nc.tensor.transpose(et_ps[:, c, :], EN[:, c, :], identb[:])
                gf = smallp.tile([128, 1], fp32, tag="gf")
                nc.scalar.copy(gf[:], qt2[:, 0:1])
                ET = ebufp.tile([128, NCH, 128], bf16, tag="et")  # [j, c, p]
                nc.vector.tensor_scalar_mul(
                    ET[:].rearrange("m c p -> m (c p)"),
                    et_ps[:].rearrange("m c p -> m (c p)"),
                    gf[:],
                )

                # ---- next batch's Wm (its inputs are already loading)
                if nxt_io is not None:
                    cur = (nxt_io, make_wm(nxt_io[1]))

                # ---- chunk loop: one N=257 bf16 matmul [EQ | ET2 | rs] per
                # chunk; plain 257-col copy drains, lag 2, 4 ACT / 4 DVE
                o_big = obufp.tile([128, NCH, 257], bf16, tag="obig")
                p24s = [None] * NCH

                def drain(c):
                    if c in ACT_DRAINS:
                        nc.scalar.copy(o_big[:, c, :], p24s[c][:, 0:257])
                    else:
                        nc.vector.tensor_copy(o_big[:, c, :], p24s[c][:, 0:257])

                for c in range(NCH):
                    p24s[c] = ps24p.tile(
                        [128, 257], fp32, tag="ps24", name=f"p24_{bi}_{c}"
                    )
                    nc.tensor.matmul(p24s[c][:], ET[:, c, :], qt2[:, 1:258])
                    if c >= 2:
                        drain(c - 2)
                drain(NCH - 2)
                drain(NCH - 1)

                # ---- single contiguous ~0.5MB store
                nc.sync.dma_start(
                    O_d.ap()[b].rearrange("(p c) e -> p c e", c=NCH), o_big[:]
                )

    nc.compile()
    return nc


def make_in_maps(C, Q, W):
    bf = ml_dtypes.bfloat16
    Cf = np.ascontiguousarray(C, dtype=np.float32)
    Qf = np.ascontiguousarray(Q, dtype=np.float32)
    Wf = np.ascontiguousarray(W, dtype=np.float32)
    g = np.exp(Qf @ Wf[D : 2 * D])                 # [B, M] fp32 on host
    Qcat = np.concatenate([g[:, :, None], Qf], axis=2).astype(bf)  # [B, M, D+1]
    Ccat = np.concatenate([Cf, np.ones((B, N, 1), np.float32)], axis=2).astype(bf)
    # [C^T | Q^T]: Tin[b, d, c*128+p] = C[b, 8p+c, d]; Tin[b, d, N+j] = Q[b, j, d]
    Cb = Cf.astype(bf).reshape(B, 128, NCH, D)     # [b, p, c, d]
    CT = np.ascontiguousarray(Cb.transpose(0, 3, 2, 1)).reshape(B, D, N)
    QT = np.ascontiguousarray(Qf.astype(bf).transpose(0, 2, 1))  # [b, d, j]
    Tin = np.concatenate([CT, QT], axis=2)         # [B, D, N+M] bf16
    return [
        {
            "Cin": Ccat[i * BPC : (i + 1) * BPC],
            "Tin": Tin[i * BPC : (i + 1) * BPC],
            "Qin": Qcat[i * BPC : (i + 1) * BPC],
            "Win": Wf,
        }
        for i in range(NCORES)
    ]


def kernel(C, Q, W):
    from concourse.bass_utils import run_bass_kernel_spmd

    if "nc" not in _CACHE:
        _CACHE["nc"] = _build_program()
    nc = _CACHE["nc"]

    in_maps = make_in_maps(C, Q, W)
    res = run_bass_kernel_spmd(nc, in_maps, core_ids=list(range(NCORES)))
    _CACHE["last_result"] = res
    Cf = np.asarray(C, dtype=np.float32)
    ab = np.concatenate([r["Out"] for r in res.results], axis=0)  # [B, N, 257] bf16
    ab = ab.astype(np.float32)
    rs = ab[:, :, 256:257]
    A = ab[:, :, 0:D] / rs
    Bm = ab[:, :, D : 2 * D] / rs
    out = np.empty((B, N, 4 * D), dtype=np.float32)
    out[:, :, 0:D] = Cf
    out[:, :, D : 2 * D] = A
    out[:, :, 2 * D : 3 * D] = Cf * A
    out[:, :, 3 * D : 4 * D] = Cf * Bm
    return out
